# revision 5
# baseline (speedup 1.0000x reference)
"""Trainium2 Bass kernel for nn_LossSoftDice (soft-dice loss over 32 samples
of 1x512x512 probability/target maps).

Strategy: pure data parallel over the batch. Each of the 8 NeuronCores gets 4
samples (each sample = 262144 f32 elements, viewed as a [128, 2048] tile).
The device computes only per-partition statistics (everything else is
O(128) work done on host during the gather/unshard step):

  inter_p[p]  = sum_f m1[p,f] * m2[p,f]          (DVE tensor_tensor_reduce)
  sum1_p[p]   = sum_f m1[p,f]                     (ACT activation-accumulate)
  sum2_p[p]   = sum_f m2[p,f]                     (ACT activation-accumulate)
  maxp[p]     = max_f m2[p,f]                     (DVE tensor_reduce)
  nsr_p[p]    = #{f : m1[p,f] > 0.5}              (DVE tensor_scalar accum)
  corrl_p[p]  = #{f : (m1[p,f] > 0.5) == (m2[p,f] == maxp[p])}
                                                  (DVE scalar_tensor_tensor)

Host combine (exact, matches the reference's acc branch):
  gmax = max_p maxp[p]
  corr = sum_p ( corrl_p[p]        if maxp[p] == gmax
                 else 2048 - nsr_p[p] )           # all GT==0 in those rows
  score = 2*(inter+1)/(sum1+sum2+1);  score = 1 where corr == 1
  loss = mean(1 - score)
"""

import os
import sys
import types

import numpy as np


def _ensure_concourse():
    try:
        import concourse.bass  # noqa: F401
    except ImportError:
        for p in ("/opt/trn_rl_repo", "/root/.axon_site/_ro/trn_rl_repo"):
            if os.path.isdir(p) and p not in sys.path:
                sys.path.insert(0, p)
        import concourse.bass  # noqa: F401


_ensure_concourse()

import concourse.bass as bass  # noqa: E402
import concourse.bacc as bacc  # noqa: E402
import concourse.tile as tile  # noqa: E402
from concourse import mybir  # noqa: E402
from concourse.bass_utils import run_bass_kernel_spmd  # noqa: E402
from concourse.vector_clock import ScopedClock  # noqa: E402

N_CORES = 8
B = 32                      # total batch
BPC = B // N_CORES          # samples per core
P = 128                     # partitions
F = 2048                    # free dim per partition (P*F = 512*512)

_MAX_WAITS_PER_INST = 1


def _patched_drain_and_barrier(self, tick_clock, wait_clock):
    """Walrus CoreV3Gen rejects CTRL instructions with >2 sem waits; the Tile
    tail drain can carry many. Split them one-per-NoOp before the drain."""
    nc = self.nc
    drain_inst = nc.sync.drain()
    wait_clock.add_sem_waits(
        drain_inst.ins, ScopedClock({None: tick_clock.global_clock})
    )
    si = drain_inst.ins.sync_info
    if si is not None and si.on_wait and len(si.on_wait) > _MAX_WAITS_PER_INST:
        waits = list(si.on_wait)
        si.on_wait = waits[:_MAX_WAITS_PER_INST]
        insts = nc.cur_bb.bb.instructions
        assert insts[-1] is drain_inst.ins
        nops = []
        for w in waits[_MAX_WAITS_PER_INST:]:
            nop_inst = nc.sync.nop(nofuse=True, hint="drain_wait_split")
            if nop_inst.ins.sync_info is None:
                nop_inst.ins.sync_info = mybir.SyncInfo(on_wait=[], on_update=[])
            nop_inst.ins.sync_info.on_wait.append(w)
            nops.append(insts.pop())
        d = insts.pop()
        insts.extend(nops)
        insts.append(d)

    nc.all_engine_barrier()
    assert self.sems is not None
    popped = nc._tile_sem_poison_stack.pop()
    assert popped is self._sem_poison
    nc.clear_and_free_semaphores(list(self.sems.allocated().values()))
    nc.all_engine_barrier()


# Bacc.compile() legalizes multi-wait instructions; drain patch not installed.


def _install_ntff_hook_module():
    """bass_utils imports antenv.axon_hooks when trace=True under axon; this
    container's antenv lacks that module. Recreate it from the boot helper."""
    if "antenv.axon_hooks" in sys.modules:
        return
    try:
        import trn_agent_boot.trn_boot as tb

        hook = tb._ntff_profile_via_ctypes("/opt/axon/libaxon_pjrt.so")
    except Exception:
        hook = None
    m = types.ModuleType("antenv.axon_hooks")
    m.get_axon_ntff_profile_hook = lambda: hook
    m.set_axon_ntff_profile_hook = lambda h: None
    sys.modules["antenv.axon_hooks"] = m


_STAT_NAMES = ("inter", "sum1", "sum2", "maxp", "nsr", "corrl")


def _build_nc():
    nc = bacc.Bacc("TRN2", debug=False)
    f32 = mybir.dt.float32
    probs = nc.dram_tensor("probs", [BPC, P, F], f32, kind="ExternalInput").ap()
    targets = nc.dram_tensor("targets", [BPC, P, F], f32, kind="ExternalInput").ap()
    outs = {
        name: nc.dram_tensor(name, [P, BPC], f32, kind="ExternalOutput").ap()
        for name in _STAT_NAMES
    }

    A = mybir.AluOpType
    with tile.TileContext(nc) as tc:
        with (
            tc.tile_pool(name="m1", bufs=BPC) as m1_pool,
            tc.tile_pool(name="m2", bufs=BPC) as m2_pool,
            tc.tile_pool(name="scr", bufs=1) as scr_pool,
            tc.tile_pool(name="sr", bufs=2) as sr_pool,
            tc.tile_pool(name="stats", bufs=1) as stats_pool,
        ):
            m1s, m2s = [], []
            for s in range(BPC):
                m1 = m1_pool.tile([P, F], f32, tag="m1")
                nc.sync.dma_start(m1[:], probs[s])
                m2 = m2_pool.tile([P, F], f32, tag="m2")
                nc.sync.dma_start(m2[:], targets[s])
                m1s.append(m1)
                m2s.append(m2)

            dve_scr = scr_pool.tile([P, F], f32, tag="dve_scr")
            act_scr = scr_pool.tile([P, F], f32, tag="act_scr")
            st = {
                name: stats_pool.tile(
                    [P, BPC], f32, tag=f"st_{name}", name=f"st_{name}"
                )
                for name in _STAT_NAMES
            }

            for s in range(BPC):
                m1, m2 = m1s[s], m2s[s]
                c = slice(s, s + 1)
                # intersection per partition (+ throwaway product tile)
                nc.vector.scalar_tensor_tensor(
                    out=dve_scr[:],
                    in0=m1[:],
                    scalar=1.0,
                    in1=m2[:],
                    op0=A.mult,
                    op1=A.mult,
                    accum_out=st["inter"][:, c],
                )
                # per-partition sums on the scalar engine
                nc.scalar.activation(
                    act_scr[:], m1[:], mybir.ActivationFunctionType.Copy,
                    accum_out=st["sum1"][:, c],
                )
                nc.scalar.activation(
                    act_scr[:], m2[:], mybir.ActivationFunctionType.Copy,
                    accum_out=st["sum2"][:, c],
                )
                # per-partition max of targets
                nc.vector.tensor_reduce(
                    st["maxp"][:, c], m2[:], mybir.AxisListType.X, A.max
                )
                # SR = m1 > 0.5 (and its count)
                sr = sr_pool.tile([P, F], f32, tag="sr")
                nc.vector.tensor_scalar(
                    sr[:], m1[:], 0.5, None, A.is_gt, A.add,
                    accum_out=st["nsr"][:, c],
                )
                # corr-local = #{(m2 == maxp) == SR}
                nc.vector.scalar_tensor_tensor(
                    out=dve_scr[:],
                    in0=m2[:],
                    scalar=st["maxp"][:, c],
                    in1=sr[:],
                    op0=A.is_equal,
                    op1=A.is_equal,
                    accum_out=st["corrl"][:, c],
                )

            for name in _STAT_NAMES:
                nc.sync.dma_start(outs[name], st[name][:])

    nc.compile()
    return nc


def _shard_inputs(probs, targets):
    probs = np.ascontiguousarray(np.asarray(probs, dtype=np.float32)).reshape(B, P, F)
    targets = np.ascontiguousarray(np.asarray(targets, dtype=np.float32)).reshape(
        B, P, F
    )
    in_maps = []
    for i in range(N_CORES):
        sl = slice(i * BPC, (i + 1) * BPC)
        in_maps.append(
            {
                "probs": np.ascontiguousarray(probs[sl]),
                "targets": np.ascontiguousarray(targets[sl]),
            }
        )
    return in_maps


def _combine(results):
    """Exact host-side combine of per-partition stats -> scalar loss."""
    inter = np.empty(B)
    sum1 = np.empty(B)
    sum2 = np.empty(B)
    corr = np.empty(B)
    for i in range(N_CORES):
        r = results[i]
        for s in range(BPC):
            b = i * BPC + s
            inter[b] = r["inter"][:, s].astype(np.float64).sum()
            sum1[b] = r["sum1"][:, s].astype(np.float64).sum()
            sum2[b] = r["sum2"][:, s].astype(np.float64).sum()
            maxp = r["maxp"][:, s]
            gmax = maxp.max()
            at_max = maxp == gmax
            corr[b] = np.where(
                at_max, r["corrl"][:, s], float(F) - r["nsr"][:, s]
            ).astype(np.float64).sum()
    score = 2.0 * (inter + 1.0) / (sum1 + sum2 + 1.0)
    score = np.where(corr == 1.0, 1.0, score)
    return np.array(np.mean(1.0 - score), dtype=np.float32)


def _run(probs, targets, trace=False, tmpdir=None):
    if trace:
        _install_ntff_hook_module()
    nc = _build_nc()
    in_maps = _shard_inputs(probs, targets)
    res = run_bass_kernel_spmd(
        nc, in_maps, list(range(N_CORES)), trace=trace, tmpdir=tmpdir
    )
    out = _combine(res.results)
    return out, res


def kernel(probs, targets):
    out, _ = _run(probs, targets)
    return out
